# revision 9
# baseline (speedup 1.0000x reference)
"""Trainium2 Bass kernel for nn_ExtractRelevantPatchesLayer.

Per-image: 64x64 avg-pool on a [1024,1024] heatmap -> top-32 of the 256
pooled values -> gather the corresponding 64x64x3 image patches.

Sharding: batch dim (16) data-parallel across 8 NeuronCores, 2 images per
core, no cross-core communication.

Bit-exactness strategy (pooled values have 1-ULP gaps, so patch selection
must reproduce jax's f32 summation order exactly):
  - jax mean = sequential f32 sum over the 64 contiguous columns, then
    sequential f32 sum over the 64 rows (verified bitwise vs jax CPU+TRN).
  - DVE tensor_reduce is sequential over the free axis (HW-verified).
  - PE transpose (identity matmul) moves f32 bitwise (HW-verified), letting
    a second DVE reduce do the row sums sequentially.
  - top-32 via 4 rounds of max8/max_index/match_replace reproduces
    jax.lax.top_k ordering including duplicate handling (HW-verified).
Only the selected patches are read from HBM (dma_gather of 192-float rows),
so HBM traffic/core is ~8 MiB heatmap + 3 MiB gather + 3 MiB store.
"""
import os
import sys

for p in ("/opt/trn_rl_repo", "/root/.axon_site/_ro/trn_rl_repo"):
    if os.path.isdir(p) and p not in sys.path:
        sys.path.append(p)

import numpy as np

import concourse.bacc as bacc
import concourse.bass_isa as bass_isa
import concourse.mybir as mybir
import concourse.tile as tile
from concourse.bass_utils import run_bass_kernel_spmd

F32 = mybir.dt.float32
I32 = mybir.dt.int32
I16 = mybir.dt.int16
U32 = mybir.dt.uint32

B_LOCAL = 2          # batches per core
N_CORES = 8
P = 64               # patch size
K = 32               # patches kept per batch
GRID = 16            # 16x16 candidate patches
NEG_FILL = -1.0e30

_CACHE: dict = {}


def _build_module():
    nc = bacc.Bacc("TRN2", target_bir_lowering=False, debug=False)

    # Local shard tensors (per core): 2 batches.
    hm_d = nc.dram_tensor("hm", [B_LOCAL * 1024, 1024], F32, kind="ExternalInput")
    img_d = nc.dram_tensor("img", [B_LOCAL * 16384, 192], F32, kind="ExternalInput")
    sel_d = nc.dram_tensor("sel", [B_LOCAL * K, P * P * 3], F32, kind="ExternalOutput")

    # Inline constants.
    ident_np = np.eye(128, dtype=np.float32)
    ident_d = nc.inline_tensor(ident_np, name="ident")
    # iotaA[p, s] = (p%16)*16 + (s%4)*256   (s = j*4 + rr_hi)
    pp, ss = np.meshgrid(np.arange(128), np.arange(128), indexing="ij")
    iota_np = ((pp % 16) * 16 + (ss % 4) * 256).astype(np.int32)
    iota_d = nc.inline_tensor(iota_np, name="iotaA")

    with tile.TileContext(nc) as tc:
        with tc.tile_pool(name="consts", bufs=1) as cpool, \
             tc.tile_pool(name="heat", bufs=4) as hpool, \
             tc.tile_pool(name="work", bufs=1) as wpool, \
             tc.tile_pool(name="gath", bufs=1) as gpool, \
             tc.tile_pool(name="dr", bufs=1, space="DRAM") as dpool, \
             tc.tile_pool(name="ps", bufs=2, space="PSUM") as ppool:

            ident = cpool.tile([128, 128], F32, tag="ident", name="ident")
            nc.sync.dma_start(ident[:], ident_d[:])
            iota = cpool.tile([128, 128], I32, tag="iota", name="iota")
            nc.sync.dma_start(iota[:], iota_d[:])

            # Column partials: P_all[b][p, t*16+gw] = seq-sum over the 64
            # cols of group gw, row (t*128+p) of batch b.
            p_all = [wpool.tile([128, 128], F32, tag=f"pall{b}", name=f"pall{b}") for b in range(B_LOCAL)]
            for t in range(16):
                ht = hpool.tile([128, 1024], F32, tag="heat", name="heat")
                nc.sync.dma_start(ht[:], hm_d[t * 128:(t + 1) * 128, :])
                nc.vector.tensor_reduce(
                    out=p_all[t // 8][:, (t % 8) * 16:((t % 8) + 1) * 16],
                    in_=ht[:].rearrange("p (g c) -> p g c", c=64),
                    axis=mybir.AxisListType.X,
                    op=mybir.AluOpType.add,
                )

            # Row sums per batch: transpose partials so each partition holds
            # one (t, gw) column of 128 row-partials, then reduce per 64.
            # Both batches share one padded sums tile (b0 cols 0:2, b1 32:34)
            # so a single second transpose serves both.
            sums = wpool.tile([128, 64], F32, tag="sums", name="sums")
            nc.vector.memset(sums[:], 0.0)
            for b in range(B_LOCAL):
                pt = ppool.tile([128, 128], F32, tag="pt", name="pt")
                nc.tensor.transpose(pt[:], p_all[b][:], ident[:])
                nc.vector.tensor_reduce(
                    out=sums[:, b * 32:b * 32 + 2],
                    in_=pt[:].rearrange("q (m r) -> q m r", r=64),
                    axis=mybir.AxisListType.X,
                    op=mybir.AluOpType.add,
                )
            pt2 = ppool.tile([64, 128], F32, tag="pt2", name="pt2")
            nc.tensor.transpose(pt2[:], sums[:], ident[:])
            s2 = wpool.tile([64, 128], F32, tag="s2", name="s2")
            nc.vector.tensor_copy(s2[0:2, :], pt2[0:2, :])
            nc.vector.tensor_copy(s2[32:34, :], pt2[32:34, :])

            for b in range(B_LOCAL):
                # Flatten to [1, 256] in n = gh*16+gw = 32t+16m+gw order via
                # two same-partition strided DMAs (one per m).
                vflat = wpool.tile([1, 256], F32, tag=f"vflat{b}", name=f"vflat{b}")
                vflat_v = vflat[:].rearrange(
                    "o (t m g) -> o m t g", t=8, m=2, g=16)
                for m in range(2):
                    nc.sync.dma_start(vflat_v[:, m], s2[b * 32 + m:b * 32 + m + 1, :])
                # Replicate to all 128 partitions.
                vrep = wpool.tile([128, 256], F32, tag=f"vrep{b}", name=f"vrep{b}")
                nc.gpsimd.partition_broadcast(vrep[:], vflat[:], channels=128)

                # Top-32, descending, jax tie order.
                idxs = wpool.tile([128, 32], U32, tag=f"idx{b}", name=f"idx{b}")
                for rnd in range(4):
                    mx = wpool.tile([128, 8], F32, tag=f"mx{b}", name=f"mx{b}")
                    nc.vector.max(out=mx[:], in_=vrep[:])
                    nc.vector.max_index(
                        out=idxs[:, rnd * 8:(rnd + 1) * 8],
                        in_max=mx[:], in_values=vrep[:])
                    nc.vector.match_replace(
                        out=vrep[:], in_to_replace=mx[:], in_values=vrep[:],
                        imm_value=NEG_FILL)

                # Gather row indices: k = rowbase + rr_hi*256 + q*16, with
                # rowbase = n + 1008*(n>>4)  (n = gh*16+gw).
                idx_i = wpool.tile([128, 32], I32, tag=f"idxi{b}", name=f"idxi{b}")
                nc.vector.tensor_copy(idx_i[:], idxs[:])
                n16 = wpool.tile([128, 32], I32, tag=f"n16{b}", name=f"n16{b}")
                nc.vector.tensor_scalar(
                    n16[:], idx_i[:], 4, None,
                    op0=mybir.AluOpType.logical_shift_right)
                rb = wpool.tile([128, 32], I32, tag=f"rb{b}", name=f"rb{b}")
                nc.vector.tensor_scalar(
                    rb[:], n16[:], 1008, None, op0=mybir.AluOpType.mult)
                nc.vector.tensor_add(rb[:], rb[:], idx_i[:])
                krows = wpool.tile([128, 128], I32, tag=f"krows{b}", name=f"krows{b}")
                nc.vector.tensor_add(
                    krows[:].rearrange("p (j h) -> p j h", h=4),
                    iota[:].rearrange("p (j h) -> p j h", h=4),
                    rb[:].to_broadcast([128, 32, 4]))
                idx16 = wpool.tile([128, 128], I16, tag=f"k16{b}", name=f"k16{b}")
                nc.vector.tensor_copy(idx16[:], krows[:])

                # Gather the 2048 patch rows (192 f32 each) of this batch.
                gath = gpool.tile([128, 16 * 192], F32, tag=f"g{b}", name=f"g{b}")
                nc.gpsimd.dma_gather(
                    out_ap=gath[:].rearrange("p (m c) -> p m c", c=192),
                    in_ap=img_d[b * 16384:(b + 1) * 16384, :],
                    idxs_ap=idx16[:],
                    num_idxs=2048,
                    num_idxs_reg=2048,
                    elem_size=192,
                    single_packet=False,
                )
                # Store: gathered row g=j*64+rr sits at [64*(j%2)+rr, j//2].
                sel_v = sel_d[:].rearrange(
                    "(bb jh jl) (r c) -> bb jl r jh c", bb=B_LOCAL, jh=16, jl=2, c=192)
                for jl in range(2):
                    nc.sync.dma_start(
                        sel_v[b, jl],
                        gath[jl * 64:(jl + 1) * 64, :].rearrange(
                            "p (m c) -> p m c", c=192),
                    )

    nc.compile()
    return nc


def _get_module():
    if "nc" not in _CACHE:
        _CACHE["nc"] = _build_module()
    return _CACHE["nc"]


LAST_RESULTS = None  # BassKernelResults of the most recent kernel() call


def kernel(heatmap, image):
    global LAST_RESULTS
    heatmap = np.ascontiguousarray(np.asarray(heatmap), dtype=np.float32)
    image = np.ascontiguousarray(np.asarray(image), dtype=np.float32)
    B = heatmap.shape[0]
    assert B == B_LOCAL * N_CORES

    nc = _get_module()
    in_maps = []
    for c in range(N_CORES):
        hm = heatmap[c * B_LOCAL:(c + 1) * B_LOCAL].reshape(B_LOCAL * 1024, 1024)
        im = image[c * B_LOCAL:(c + 1) * B_LOCAL].reshape(B_LOCAL * 16384, 192)
        in_maps.append({"hm": hm, "img": im})

    trace = os.environ.get("KERNEL_PROFILE", "") == "1"
    try:
        res = run_bass_kernel_spmd(
            nc, in_maps, core_ids=list(range(N_CORES)), trace=trace)
    except ModuleNotFoundError:
        # NTFF profiling hook unavailable in this environment
        res = run_bass_kernel_spmd(
            nc, in_maps, core_ids=list(range(N_CORES)), trace=False)
    LAST_RESULTS = res
    out = np.concatenate(
        [res.results[c]["sel"].reshape(B_LOCAL * K, P, P, 3) for c in range(N_CORES)],
        axis=0)
    return out


# revision 23
# speedup vs baseline: 87556.9865x; 87556.9865x over previous
"""Trainium2 Bass kernel for nn_ExtractRelevantPatchesLayer.

Per-image: 64x64 avg-pool on a [1024,1024] heatmap -> top-32 of the 256
pooled values -> gather the corresponding 64x64x3 image patches.

Sharding: batch dim (16) data-parallel across 8 NeuronCores, 2 images per
core, no cross-core communication.

Bit-exactness strategy (pooled values have 1-ULP gaps, so patch selection
must reproduce jax's f32 summation order exactly):
  - jax mean = sequential f32 sum over the 64 contiguous columns, then
    sequential f32 sum over the 64 rows (verified bitwise vs jax CPU+TRN).
  - DVE tensor_reduce is sequential over the free axis (HW-verified).
  - PE transpose (identity matmul) moves f32 bitwise (HW-verified), letting
    a second DVE reduce do the row sums sequentially.
  - top-32 via 4 rounds of max8/max_index/match_replace reproduces
    jax.lax.top_k ordering including duplicate handling (HW-verified).
Only the selected patches are read from HBM (dma_gather of 192-float rows),
so HBM traffic/core is ~8 MiB heatmap + 3 MiB gather + 3 MiB store.
"""
import os
import sys

for p in ("/opt/trn_rl_repo", "/root/.axon_site/_ro/trn_rl_repo"):
    if os.path.isdir(p) and p not in sys.path:
        sys.path.append(p)

import numpy as np

import concourse.bacc as bacc
import concourse.bass_isa as bass_isa
import concourse.mybir as mybir
import concourse.tile as tile
from concourse.tile_rust import add_dep_helper as _add_dep
from concourse.bass_utils import run_bass_kernel_spmd

F32 = mybir.dt.float32
I32 = mybir.dt.int32
I16 = mybir.dt.int16
U32 = mybir.dt.uint32

B_LOCAL = 2          # batches per core
N_CORES = 8
P = 64               # patch size
K = 32               # patches kept per batch
GRID = 16            # 16x16 candidate patches
NEG_FILL = -1.0e30

_CACHE: dict = {}


def _build_module():
    nc = bacc.Bacc("TRN2", target_bir_lowering=False, debug=False)

    # Local shard tensors (per core): 2 batches.
    hm_d = nc.dram_tensor("hm", [B_LOCAL * 1024, 1024], F32, kind="ExternalInput")
    img_d = nc.dram_tensor("img", [B_LOCAL * 16384, 192], F32, kind="ExternalInput")
    sel_d = nc.dram_tensor("sel", [B_LOCAL * K, P * P * 3], F32, kind="ExternalOutput")

    # Inline constants.
    ident_d = nc.inline_tensor(np.eye(128, dtype=np.float32), name="ident")
    ones_d = nc.inline_tensor(np.ones((128, 128), np.float32), name="ones")
    pp, ss = np.meshgrid(np.arange(128), np.arange(128), indexing="ij")
    # iotaA[p, s] = (p%16)*16 + (s%4)*256   (s = j*4 + rr_hi)
    iota_d = nc.inline_tensor(
        ((pp % 16) * 16 + (ss % 4) * 256).astype(np.float32), name="iotaA")
    # per-partition candidate index n(q, m) = 32*(q//16) + 16*m + q%16
    q = np.arange(128)
    n_qm = (32 * (q[:, None] // 16) + 16 * np.arange(2)[None, :]
            + (q[:, None] % 16))                        # [128, 2]
    # ltmask_m[q, f] = 1.0 if f < n(q, m)  (stable-rank tie term)
    f = np.arange(256)
    lt_np = (f[None, None, :] < n_qm[:, :, None]).astype(np.float32)  # [128,2,256]
    lt0_d = nc.inline_tensor(lt_np[:, 0, :].copy(), name="lt0")
    lt1_d = nc.inline_tensor(lt_np[:, 1, :].copy(), name="lt1")
    # rbase[q, m] = image row-block base of patch n(q, m)
    rbase_np = (n_qm + 1008 * (n_qm >> 4)).astype(np.float32)         # [128, 2]
    rbase_d = nc.inline_tensor(rbase_np, name="rbase")
    # jconst[p, j] = j  for slot-match
    jconst_d = nc.inline_tensor(
        np.tile(np.arange(32, dtype=np.float32), (128, 1)), name="jconst")

    with tile.TileContext(nc) as tc:
        with tc.tile_pool(name="consts", bufs=1) as cpool, \
             tc.tile_pool(name="heat", bufs=6) as hpool, \
             tc.tile_pool(name="work", bufs=1) as wpool, \
             tc.tile_pool(name="gath", bufs=1) as gpool, \
             tc.tile_pool(name="dr", bufs=1, space="DRAM") as dpool, \
             tc.tile_pool(name="ps", bufs=2, space="PSUM") as ppool:

            ident = cpool.tile([128, 128], F32, tag="ident", name="ident")
            nc.scalar.dma_start(ident[:], ident_d[:])
            ones = cpool.tile([128, 128], F32, tag="ones", name="ones")
            nc.scalar.dma_start(ones[:], ones_d[:])
            iota = cpool.tile([128, 128], F32, tag="iota", name="iota")
            nc.scalar.dma_start(iota[:], iota_d[:])
            lts = cpool.tile([128, 512], F32, tag="lts", name="lts")
            nc.scalar.dma_start(lts[:, 0:256], lt0_d[:])
            nc.scalar.dma_start(lts[:, 256:512], lt1_d[:])
            rbase = cpool.tile([128, 2], F32, tag="rbase", name="rbase")
            nc.scalar.dma_start(rbase[:], rbase_d[:])
            jconst = cpool.tile([128, 32], F32, tag="jconst", name="jconst")
            nc.scalar.dma_start(jconst[:], jconst_d[:])

            # Column partials: P_all[b][p, t*16+gw] = seq-sum over the 64
            # cols of group gw, row (t*128+p) of batch b.
            p_all = [wpool.tile([128, 128], F32, tag=f"pall{b}", name=f"pall{b}") for b in range(B_LOCAL)]

            def load_and_reduce(t):
                ht = hpool.tile([128, 1024], F32, tag="heat", name="heat")
                nc.sync.dma_start(ht[:], hm_d[t * 128:(t + 1) * 128, :])
                red = nc.vector.tensor_reduce(
                    out=p_all[t // 8][:, (t % 8) * 16:((t % 8) + 1) * 16],
                    in_=ht[:].rearrange("p (g c) -> p g c", c=64),
                    axis=mybir.AxisListType.X,
                    op=mybir.AluOpType.add,
                )
                return red

            def sums_to_vrep(b):
                # Row sums: transpose partials so each partition holds one
                # (t, gw) column of 128 row-partials, then reduce per 64.
                pt = ppool.tile([128, 128], F32, tag="pt", name="pt")
                nc.tensor.transpose(pt[:], p_all[b][:], ident[:])
                # Padded to 32 free elems so the second PE transpose is legal.
                sums = wpool.tile([128, 32], F32, tag=f"sums{b}", name=f"sums{b}")
                nc.gpsimd.memset(sums[:], 0.0)
                nc.vector.tensor_reduce(
                    out=sums[:, 0:2],
                    in_=pt[:].rearrange("q (m r) -> q m r", r=64),
                    axis=mybir.AxisListType.X,
                    op=mybir.AluOpType.add,
                )
                # vrep[p, n] = pooled sum of patch n (= 32t+16m+g), on every
                # partition: transpose -> rows m, per-m partition_broadcast of
                # the contiguous (t, g) row, then DVE strided interleave.
                # No DMA anywhere in this chain.
                pt2 = ppool.tile([32, 128], F32, tag="pt2", name="pt2")
                nc.tensor.transpose(pt2[:], sums[:], ident[:])
                s2 = wpool.tile([32, 128], F32, tag=f"s2{b}", name=f"s2{b}")
                nc.vector.tensor_copy(s2[:], pt2[:])
                # partition_broadcast sources must start at partition 0, so
                # stream_shuffle row m=1 up to partition 0 of a second tile.
                s2s = wpool.tile([32, 128], F32, tag=f"s2s{b}", name=f"s2s{b}")
                nc.vector.stream_shuffle(
                    s2s[0:32, :], s2[0:32, :], mask=[1] + list(range(1, 32)))
                vrep = wpool.tile([128, 256], F32, tag=f"vrep{b}", name=f"vrep{b}")
                vrep_v = vrep[:].rearrange("p (t m g) -> p t m g", t=8, m=2, g=16)
                for m, src in ((0, s2), (1, s2s)):
                    half = wpool.tile(
                        [128, 128], F32, tag=f"half{b}{m}", name=f"half{b}{m}")
                    nc.gpsimd.partition_broadcast(
                        half[:], src[0:1, :], channels=128)
                    nc.gpsimd.tensor_copy(
                        vrep_v[:, :, m],
                        half[:].rearrange("p (t g) -> p t g", g=16))
                return vrep, sums

            def batch_tail(b, vrep, sums):
                # Stable rank of each candidate (q, m) against all 256 pooled
                # sums: rank = #{v > v_n} + #{ties at lower n}. Matches
                # jax.lax.top_k ordering exactly (all-integer f32 math).
                rk = wpool.tile([128, 2], F32, tag=f"rk{b}", name=f"rk{b}")
                r2 = wpool.tile([128, 2], F32, tag=f"r2{b}", name=f"r2{b}")
                scratch = wpool.tile(
                    [128, 256], F32, tag=f"scr{b}", name=f"scr{b}")
                for m in range(2):
                    nc.vector.tensor_scalar(
                        scratch[:], vrep[:], sums[:, m:m + 1], 0.0,
                        op0=mybir.AluOpType.is_gt,
                        op1=mybir.AluOpType.add,
                        accum_out=rk[:, m:m + 1])
                    nc.vector.scalar_tensor_tensor(
                        out=scratch[:], in0=vrep[:], scalar=sums[:, m:m + 1],
                        in1=lts[:, m * 256:(m + 1) * 256],
                        op0=mybir.AluOpType.is_equal,
                        op1=mybir.AluOpType.mult,
                        accum_out=r2[:, m:m + 1])
                nc.vector.tensor_add(rk[:], rk[:], r2[:])

                # One-hot slot matrix scaled by rbase, then ones.T @ ZR
                # replicates the per-slot row-base across all partitions.
                zr = wpool.tile([128, 64], F32, tag=f"zr{b}", name=f"zr{b}")
                nc.vector.tensor_scalar(
                    zr[:, 0:32], jconst[:], rk[:, 0:1], None,
                    op0=mybir.AluOpType.is_equal)
                nc.vector.tensor_scalar(
                    zr[:, 32:64], jconst[:], rk[:, 1:2], None,
                    op0=mybir.AluOpType.is_equal)
                nc.vector.tensor_scalar(
                    zr[:, 0:32], zr[:, 0:32], rbase[:, 0:1], None,
                    op0=mybir.AluOpType.mult)
                nc.vector.scalar_tensor_tensor(
                    out=zr[:, 0:32], in0=zr[:, 32:64], scalar=rbase[:, 1:2],
                    in1=zr[:, 0:32],
                    op0=mybir.AluOpType.mult, op1=mybir.AluOpType.add)
                rbs = ppool.tile([128, 32], F32, tag="rbs", name="rbs")
                nc.tensor.matmul(
                    out=rbs[:], lhsT=ones[:], rhs=zr[:, 0:32],
                    start=True, stop=True)

                # k = rbs[slot] + rr_hi*256 + q*16, converted to int16.
                krows = wpool.tile(
                    [128, 128], F32, tag=f"krows{b}", name=f"krows{b}")
                kr_inst = nc.vector.tensor_add(
                    krows[:].rearrange("p (j h) -> p j h", h=4),
                    iota[:].rearrange("p (j h) -> p j h", h=4),
                    rbs[:].to_broadcast([128, 32, 4]))
                idx16 = wpool.tile([128, 128], I16, tag=f"k16{b}", name=f"k16{b}")
                nc.gpsimd.tensor_copy(idx16[:], krows[:])

                # Gather the 2048 patch rows (192 f32 each) of this batch.
                gath = gpool.tile([128, 16 * 192], F32, tag=f"g{b}", name=f"g{b}")
                nc.gpsimd.dma_gather(
                    out_ap=gath[:].rearrange("p (m c) -> p m c", c=192),
                    in_ap=img_d[b * 16384:(b + 1) * 16384, :],
                    idxs_ap=idx16[:],
                    num_idxs=2048,
                    num_idxs_reg=2048,
                    elem_size=192,
                    single_packet=False,
                )
                # Store: gathered row g=j*64+rr sits at [64*(j%2)+rr, j//2].
                sel_v = sel_d[:].rearrange(
                    "(bb jh jl) (r c) -> bb jl r jh c", bb=B_LOCAL, jh=16, jl=2, c=192)
                for jl in range(2):
                    nc.sync.dma_start(
                        sel_v[b, jl],
                        gath[jl * 64:(jl + 1) * 64, :].rearrange(
                            "p (m c) -> p m c", c=192),
                    )
                return kr_inst

            # Emission order = scheduler priority: batch 0's entire tail
            # outranks batch 1's loads/reduces, so b0's gather DMA is ready
            # the moment the heatmap stream drains.
            for t in range(8):
                load_and_reduce(t)
            kr0 = batch_tail(0, *sums_to_vrep(0))
            late_reds = []
            for t in range(8, 16):
                red = load_and_reduce(t)
                if t >= 12:
                    late_reds.append(red)
            # Keep DVE clear for batch 0's rank chain: the last four batch-1
            # reduces wait until b0's final DVE op so its gather can be
            # enqueued the moment the heatmap stream drains.
            for red in late_reds:
                _add_dep(red.ins, kr0.ins,
                         reason="pipeline: late b1 reduces yield to b0 rank chain")
            batch_tail(1, *sums_to_vrep(1))

    nc.compile()
    return nc


def _get_module():
    if "nc" not in _CACHE:
        _CACHE["nc"] = _build_module()
    return _CACHE["nc"]


LAST_RESULTS = None  # BassKernelResults of the most recent kernel() call


def kernel(heatmap, image):
    global LAST_RESULTS
    heatmap = np.ascontiguousarray(np.asarray(heatmap), dtype=np.float32)
    image = np.ascontiguousarray(np.asarray(image), dtype=np.float32)
    B = heatmap.shape[0]
    assert B == B_LOCAL * N_CORES

    nc = _get_module()
    in_maps = []
    for c in range(N_CORES):
        hm = heatmap[c * B_LOCAL:(c + 1) * B_LOCAL].reshape(B_LOCAL * 1024, 1024)
        im = image[c * B_LOCAL:(c + 1) * B_LOCAL].reshape(B_LOCAL * 16384, 192)
        in_maps.append({"hm": hm, "img": im})

    trace = os.environ.get("KERNEL_PROFILE", "") == "1"
    try:
        res = run_bass_kernel_spmd(
            nc, in_maps, core_ids=list(range(N_CORES)), trace=trace)
    except ModuleNotFoundError:
        # NTFF profiling hook unavailable in this environment
        res = run_bass_kernel_spmd(
            nc, in_maps, core_ids=list(range(N_CORES)), trace=False)
    LAST_RESULTS = res
    out = np.concatenate(
        [res.results[c]["sel"].reshape(B_LOCAL * K, P, P, 3) for c in range(N_CORES)],
        axis=0)
    return out
